# revision 8
# baseline (speedup 1.0000x reference)
"""Bilinear sampling kernel v2 for Trainium2 (Bass/Tile), data-parallel 8 cores.

Same math as the baseline kernel (2-parity 256B-slot table + per-pixel SWDGE
gather + DVE combine), restructured for overlap:

 - Per-image pipeline: build(b+1) runs on Sync/Scalar/DVE while the gather
   stream of image b occupies GPSIMD.  No global phase split.
 - No DRAM round trips for weights (wx1/wy1/m1 stay in SBUF) or idxs.
 - The idx wrap widx[r, 8n+m] = k16[16m+r, n] is built as 8 contiguous
   SBUF->SBUF partition-rebase DMAs (k16[16m:16m+16,:] -> K2[0:16, 392m:])
   plus one DVE free-dim interleave copy, instead of the baseline's 8
   stride-8 int16 DMAs (2-byte descriptors, ~40us each on the Sync ring).
 - GCHUNKS=28 (1792-pixel gather chunks, queues round-robin x4): small
   chunks keep each SWDGE ring under its 1024-descriptor carveout so Q7
   descriptor generation never stalls on ring-full; fine-grained combine
   groups (CGROUP chunks each) release gather tiles quickly.
 - Steady state uses only 1-port DVE ops: floor via the 2^23 RNE shifter
   trick, i16 idx extraction from the low half of (k + 2^23) bit patterns
   (ACT compaction copy).  DVE casts/copies/tensor_scalar are 2-port ops
   that would lock GPSIMD out of the shared SBUF port and stall SWDGE
   descriptor generation.
"""

import numpy as np

import concourse.bacc as bacc
import concourse.bass as bass
import concourse.mybir as mybir
from concourse.tile import TileContext

F32 = mybir.dt.float32
I32 = mybir.dt.int32
I16 = mybir.dt.int16
OP = mybir.AluOpType

H = W = 224
NIMG = 16
NCORES = 8
NPIX = H * W          # 50176
NPP = NPIX // 128     # 392
FROWS = H - 1         # 223
SLOTS_PER_ROW = W // 2
NSLOT = FROWS * SLOTS_PER_ROW  # 24976

GCHUNKS = 28     # gather instructions per image (gen/drain granularity)
CGROUP = 2       # gather chunks per combine/store group
GTH_BUFS = 8     # combine-group tiles in flight


def set_knobs(gchunks=None, gth_bufs=None, cgroup=None):
    global GCHUNKS, GTH_BUFS, CGROUP
    if gchunks is not None:
        GCHUNKS = gchunks
    if gth_bufs is not None:
        GTH_BUFS = gth_bufs
    if cgroup is not None:
        CGROUP = cgroup


def build_module(n_imgs: int = NIMG) -> bass.Bass:
    idxc = NPIX // GCHUNKS
    cj = idxc // 128
    nc = bacc.Bacc(num_swdge_queues=4)

    x_t = nc.dram_tensor("x", [n_imgs, H, W, 5], F32, kind="ExternalInput")
    out_t = nc.dram_tensor("out", [n_imgs, H, W, 3], F32, kind="ExternalOutput")

    x_flat = x_t[:].rearrange("n h w c -> n (h w c)")
    out_flat = out_t[:].rearrange("n h w c -> n (h w c)").rearrange(
        "n (p f) -> n p f", p=128
    )

    with TileContext(nc) as tc:
        with (
            tc.tile_pool(name="raw", bufs=3) as raw_pool,
            tc.tile_pool(name="mid", bufs=1) as mid_pool,
            tc.tile_pool(name="slotp", bufs=2) as slot_pool,
            tc.tile_pool(name="tmp", bufs=1) as tmp_pool,
            tc.tile_pool(name="wv", bufs=2) as wv_pool,
            tc.tile_pool(name="k16p", bufs=2) as k16_pool,
            tc.tile_pool(name="k2p", bufs=2) as k2_pool,
            tc.tile_pool(name="widxp", bufs=2) as widx_pool,
            tc.tile_pool(name="wgt", bufs=2) as wgt_pool,
            tc.tile_pool(name="gth", bufs=GTH_BUFS) as gth_pool,
            tc.tile_pool(name="accp", bufs=3) as acc_pool,
            tc.tile_pool(name="dram", bufs=1, space="DRAM") as dram_pool,
        ):
            state = {}
            imf_all = dram_pool.tile([H * n_imgs, W * 3], F32, tag="imf")
            slotd_all = dram_pool.tile([NSLOT * n_imgs, 64], F32, tag="slotd")

            # Const tiles (built once; tensor_scalar and 2-port-capable DVE
            # ops are banned from the steady state — they can lock GPSIMD
            # out of the shared SBUF port and stall SWDGE descriptor gen).
            consts = tmp_pool.tile([128, 3 * NPP], F32, tag="consts")
            ones = consts[:, 0:NPP]
            half = consts[:, NPP:2 * NPP]
            big = consts[:, 2 * NPP:3 * NPP]   # 2^23: RNE rounding shifter
            nc.vector.memset(ones, 1.0)
            nc.vector.memset(half, 0.5)
            nc.vector.memset(big, 8388608.0)

            raws = {}

            def emit_raw(b):
                """Prefetch the raw image (sync ring carries only these)."""
                raw = raw_pool.tile([128, NPP * 5], F32)
                nc.sync.dma_start(
                    out=raw[:], in_=x_flat[b].rearrange("(p f) -> p f", p=128)
                )
                raws[b] = raw

            def emit_build(b):
                """Slot table + idxs + weights (scalar DMA + ACT + DVE)."""
                raw = raws.pop(b)
                rawv = raw[:].rearrange("p (q c) -> p q c", c=5)
                x_ap = rawv[:, :, 3:4].rearrange("p q c -> p (q c)")
                y_ap = rawv[:, :, 4:5].rearrange("p q c -> p (q c)")

                # --- floors, slot ids, weights.  Cast-free: DVE casts /
                # copies / tensor_scalar are 2-port ops that mutually block
                # GPSIMD's SWDGE descriptor generation, so floor is done
                # with the 2^23 RNE shifter trick (pure tensor_tensor) and
                # the i16 idx is the low half of (k + 2^23)'s bit pattern.
                def floor_tt(src_ap, nm):
                    r = tmp_pool.tile([128, NPP], F32, tag=f"r{nm}")
                    nc.vector.tensor_tensor(out=r[:], in0=src_ap, in1=big,
                                            op=OP.add)
                    nc.vector.tensor_tensor(out=r[:], in0=r[:], in1=big,
                                            op=OP.subtract)
                    gt = tmp_pool.tile([128, NPP], F32, tag=f"g{nm}")
                    nc.vector.tensor_tensor(out=gt[:], in0=r[:], in1=src_ap,
                                            op=OP.is_gt)
                    nc.vector.tensor_tensor(out=r[:], in0=r[:], in1=gt[:],
                                            op=OP.subtract)
                    return r

                fxf = floor_tt(x_ap, "x")
                fyf = floor_tt(y_ap, "y")

                wv = wv_pool.tile([128, 3 * NPP], F32, tag="wv")
                wx1 = wv[:, 0:NPP]
                wy1 = wv[:, NPP:2 * NPP]
                m1 = wv[:, 2 * NPP:3 * NPP]
                nc.vector.tensor_tensor(out=wx1, in0=x_ap, in1=fxf[:],
                                        op=OP.subtract)
                nc.vector.tensor_tensor(out=wy1, in0=y_ap, in1=fyf[:],
                                        op=OP.subtract)
                # fl2 = floor(fx/2);  sel = fx - 2*fl2;  k = fy*112 + fl2
                fh = tmp_pool.tile([128, NPP], F32, tag="fh")
                nc.vector.tensor_tensor(out=fh[:], in0=fxf[:], in1=half,
                                        op=OP.mult)
                fl2 = floor_tt(fh[:], "h")
                nc.vector.scalar_tensor_tensor(
                    out=m1, in0=fl2[:], scalar=-2.0, in1=fxf[:],
                    op0=OP.mult, op1=OP.add,
                )
                kf = tmp_pool.tile([128, NPP], F32, tag="kf")
                nc.vector.scalar_tensor_tensor(
                    out=kf[:], in0=fyf[:], scalar=float(W // 2), in1=fl2[:],
                    op0=OP.mult, op1=OP.add,
                )
                # kb = kf + 2^23: f32 bits 0x4B00_0000 | k, so the low i16
                # of each word IS k.  Compact on ACT (same-dtype copy).
                nc.vector.tensor_tensor(out=kf[:], in0=kf[:], in1=big,
                                        op=OP.add)
                kb16 = kf[:].bitcast(I16).rearrange("p (n two) -> p n two",
                                                    two=2)
                k16 = k16_pool.tile([128, NPP], I16, tag="k16")
                nc.scalar.copy(
                    out=k16[:].rearrange("p (n one) -> p n one", one=1),
                    in_=kb16[:, :, 0:1],
                )

                # --- idx wrap: widx[r, 8n+m] = k16[16m+r, n] ---
                # step 1: partition rebase, 8 contiguous SBUF->SBUF DMAs
                k2 = k2_pool.tile([128, NPP * 8], I16, tag="k2")
                for m in range(8):
                    nc.sync.dma_start(
                        out=k2[0:16, NPP * m:NPP * (m + 1)],
                        in_=k16[16 * m:16 * (m + 1), :],
                    )
                # step 2: free-dim interleave on DVE (lane-local)
                widxs = widx_pool.tile([128, NPP * 8], I16, tag="widxs")
                k2v = k2[0:16, :].rearrange("p (m n) -> p m n", m=8)
                wxv = widxs[0:16, :].rearrange("p (n m) -> p m n", m=8)
                nc.scalar.copy(out=wxv, in_=k2v)
                # step 3: replicate to all 8 core groups
                for g in (1, 2, 4):
                    nc.sync.dma_start(
                        out=widxs[16 * g:16 * min(2 * g, 8), :],
                        in_=widxs[0:16 * min(g, 8 - g), :],
                    )

                # --- image rows to DRAM; row r = imf[r], 672 f32 ---
                img_rows = mid_pool.tile([128, NPP * 3], F32, tag="deint")
                nc.scalar.copy(
                    out=img_rows[:].rearrange("p (q c) -> p q c", c=3),
                    in_=rawv[:, :, 0:3],
                )
                imf = imf_all[H * b:H * (b + 1), :]
                nc.scalar.dma_start(
                    out=imf.rearrange("h f -> (h f)").rearrange(
                        "(p f) -> p f", p=128
                    ),
                    in_=img_rows[:],
                )

                # --- 256B-slot table (2-parity, as baseline) ---
                slotd = slotd_all[NSLOT * b:NSLOT * (b + 1), :]
                slotd_rows = slotd.rearrange(
                    "(r s) w -> r (s w)", s=SLOTS_PER_ROW
                )
                for h in range(2):
                    r0 = 128 * h
                    nrow = 128 if h == 0 else FROWS - 128
                    pp = mid_pool.tile([128, 1344], F32, tag="pp")
                    nc.scalar.dma_start(out=pp[0:nrow, 0:672],
                                        in_=imf[r0:r0 + nrow])
                    nc.scalar.dma_start(
                        out=pp[0:nrow, 672:1344], in_=imf[r0 + 1:r0 + nrow + 1]
                    )
                    frow = mid_pool.tile([128, 1350], F32, tag="frow")
                    nc.scalar.copy(
                        out=frow[0:nrow, 0:1344].rearrange(
                            "p (xx k c) -> p xx k c", k=2, c=3
                        ),
                        in_=pp[0:nrow].rearrange(
                            "p (k xx c) -> p xx k c", k=2, c=3
                        ),
                    )
                    nc.scalar.memzero(frow[0:nrow, 1344:1350])
                    half_s = SLOTS_PER_ROW // 2
                    slotd_cols = slotd_rows[r0:r0 + nrow].rearrange(
                        "r (s w) -> r s w", w=64
                    )
                    for sc in range(2):
                        slotbuf = slot_pool.tile([128, half_s * 64], F32,
                                                 tag="slotbuf")
                        fr_ap = frow[0:nrow]
                        slot_src = bass.AP(
                            fr_ap.tensor,
                            fr_ap.offset + sc * half_s * 12,
                            [list(fr_ap.ap[0]), [12, half_s], [1, 18]],
                        )
                        nc.scalar.copy(
                            out=slotbuf[0:nrow].rearrange(
                                "p (s w) -> p s w", w=64
                            )[:, :, 0:18],
                            in_=slot_src,
                        )
                        nc.scalar.dma_start(
                            out=slotd_cols[:, sc * half_s:(sc + 1) * half_s, :],
                            in_=slotbuf[0:nrow],
                        )

                # --- weight expansion (DVE, tensor_tensor only) ---
                wx0 = tmp_pool.tile([128, NPP], F32, tag="wx0")
                nc.vector.tensor_tensor(out=wx0[:], in0=ones, in1=wx1,
                                        op=OP.subtract)
                wy0 = tmp_pool.tile([128, NPP], F32, tag="wy0")
                nc.vector.tensor_tensor(out=wy0[:], in0=ones, in1=wy1,
                                        op=OP.subtract)
                m0 = tmp_pool.tile([128, NPP], F32, tag="m0")
                nc.vector.tensor_tensor(out=m0[:], in0=ones, in1=m1,
                                        op=OP.subtract)
                u0 = tmp_pool.tile([128, NPP], F32, tag="u0")
                nc.vector.tensor_tensor(out=u0[:], in0=m0[:], in1=wx0[:],
                                        op=OP.mult)
                u2 = tmp_pool.tile([128, NPP], F32, tag="u2")
                nc.vector.tensor_tensor(out=u2[:], in0=m1, in1=wx1,
                                        op=OP.mult)
                u1a = tmp_pool.tile([128, NPP], F32, tag="rh")
                nc.vector.tensor_tensor(out=u1a[:], in0=m0[:], in1=wx1,
                                        op=OP.mult)
                u1b = tmp_pool.tile([128, NPP], F32, tag="gh")
                nc.vector.tensor_tensor(out=u1b[:], in0=m1, in1=wx0[:],
                                        op=OP.mult)
                u1 = tmp_pool.tile([128, NPP], F32, tag="u1")
                nc.vector.tensor_tensor(out=u1[:], in0=u1a[:], in1=u1b[:],
                                        op=OP.add)
                wmat = []
                for v, uv in enumerate((u0, u1, u2)):
                    row = []
                    for ky, wyk in enumerate((wy0, wy1)):
                        wt = wgt_pool.tile([128, NPP], F32, tag=f"w{v}{ky}")
                        nc.vector.tensor_tensor(out=wt[:], in0=uv[:],
                                                in1=wyk[:], op=OP.mult)
                        row.append(wt)
                    wmat.append(row)
                state[b] = {"slotd": slotd, "widxs": widxs, "wmat": wmat}

            def emit_gather(b):
                st = state[b]
                slotd, widxs, wmat = st["slotd"], st["widxs"], st["wmat"]
                ngrp = GCHUNKS // CGROUP
                gj = cj * CGROUP  # slots per combine group
                for grp in range(ngrp):
                    gth = gth_pool.tile([128, gj * 64], F32, tag="gth")
                    for sub in range(CGROUP):
                        ck = grp * CGROUP + sub
                        nc.gpsimd.dma_gather(
                            out_ap=gth[:, cj * 64 * sub:cj * 64 * (sub + 1)]
                            .rearrange("p (n w) -> p n w", w=64),
                            in_ap=slotd,
                            idxs_ap=widxs[:, (idxc // 16) * ck:
                                          (idxc // 16) * (ck + 1)],
                            num_idxs=idxc,
                            num_idxs_reg=idxc,
                            elem_size=64,
                            single_packet=False,
                            queue_num=ck % 4,
                        )
                    gv = gth[:].rearrange("p (n w) -> p n w", w=64)
                    acc = acc_pool.tile([128, gj * 3], F32, tag="acc")
                    accv = acc[:].rearrange("p (q c) -> p q c", c=3)
                    tmps = []
                    for j in range(3):
                        tmps.append(acc_pool.tile([128, gj * 3], F32,
                                                  name=f"ctmp{j}",
                                                  tag=f"tmp{j}"))
                    tmpvs = []
                    for t in tmps:
                        tmpvs.append(t[:].rearrange("p (q c) -> p q c", c=3))

                    def wap_of(v, ky):
                        return (
                            wmat[v][ky][:, gj * grp: gj * (grp + 1)]
                            .unsqueeze(2)
                            .broadcast_to([128, gj, 3])
                        )

                    def mul(dst, v, ky):
                        off = 6 * v + 3 * ky
                        nc.vector.tensor_tensor(
                            out=dst, in0=gv[:, :, off:off + 3],
                            in1=wap_of(v, ky), op=OP.mult,
                        )

                    # front-load the 6 gth reads so the gather tile is
                    # released as early as possible (WAR on gth gates the
                    # next gather group)
                    mul(accv, 0, 0)
                    mul(tmpvs[0], 0, 1)
                    mul(tmpvs[1], 1, 0)
                    mul(tmpvs[2], 1, 1)
                    nc.vector.tensor_tensor(out=accv, in0=accv,
                                            in1=tmpvs[0], op=OP.add)
                    mul(tmpvs[0], 2, 0)
                    nc.vector.tensor_tensor(out=accv, in0=accv,
                                            in1=tmpvs[1], op=OP.add)
                    mul(tmpvs[1], 2, 1)
                    nc.vector.tensor_tensor(out=accv, in0=accv,
                                            in1=tmpvs[2], op=OP.add)
                    nc.vector.tensor_tensor(out=accv, in0=accv,
                                            in1=tmpvs[0], op=OP.add)
                    nc.vector.tensor_tensor(out=accv, in0=accv,
                                            in1=tmpvs[1], op=OP.add)
                    nc.sync.dma_start(
                        out=out_flat[b][:, gj * 3 * grp: gj * 3 * (grp + 1)],
                        in_=acc[:],
                    )
                del state[b]

            emit_raw(0)
            if n_imgs > 1:
                emit_raw(1)
            emit_build(0)
            for b in range(n_imgs):
                if b + 2 < n_imgs:
                    emit_raw(b + 2)
                if b + 1 < n_imgs:
                    emit_build(b + 1)
                emit_gather(b)

    nc.compile()
    return nc


def kernel(x: np.ndarray) -> np.ndarray:
    """Full-input entry point: shards the batch over 8 NeuronCores."""
    from concourse import bass_utils

    B = x.shape[0]
    assert x.shape == (B, H, W, 5) and B % NCORES == 0
    per = B // NCORES
    nc = build_module(per)
    in_maps = [
        {"x": np.ascontiguousarray(x[c * per:(c + 1) * per])}
        for c in range(NCORES)
    ]
    res = bass_utils.run_bass_kernel_spmd(nc, in_maps, core_ids=list(range(NCORES)))
    out = np.concatenate([res.results[c]["out"] for c in range(NCORES)], axis=0)
    return out


# revision 9
# speedup vs baseline: 1.4079x; 1.4079x over previous
"""Bilinear sampling kernel v2 for Trainium2 (Bass/Tile), data-parallel 8 cores.

Same math as the baseline kernel (2-parity 256B-slot table + per-pixel SWDGE
gather + DVE combine), restructured for overlap:

 - Per-image pipeline: build(b+1) runs on Sync/Scalar/DVE while the gather
   stream of image b occupies GPSIMD.  No global phase split.
 - No DRAM round trips for weights (wx1/wy1/m1 stay in SBUF) or idxs.
 - The idx wrap widx[r, 8n+m] = k16[16m+r, n] is built as 8 contiguous
   SBUF->SBUF partition-rebase DMAs (k16[16m:16m+16,:] -> K2[0:16, 392m:])
   plus one DVE free-dim interleave copy, instead of the baseline's 8
   stride-8 int16 DMAs (2-byte descriptors, ~40us each on the Sync ring).
 - GCHUNKS=28 (1792-pixel gather chunks, queues round-robin x4): small
   chunks keep each SWDGE ring under its 1024-descriptor carveout so Q7
   descriptor generation never stalls on ring-full; fine-grained combine
   groups (CGROUP chunks each) release gather tiles quickly.
 - Steady state uses only 1-port DVE ops: floor via the 2^23 RNE shifter
   trick, i16 idx extraction from the low half of (k + 2^23) bit patterns
   (ACT compaction copy).  DVE casts/copies/tensor_scalar are 2-port ops
   that would lock GPSIMD out of the shared SBUF port and stall SWDGE
   descriptor generation.
"""

import numpy as np

import concourse.bacc as bacc
import concourse.bass as bass
import concourse.mybir as mybir
from concourse.tile import TileContext

F32 = mybir.dt.float32
I32 = mybir.dt.int32
I16 = mybir.dt.int16
OP = mybir.AluOpType

H = W = 224
NIMG = 16
NCORES = 8
NPIX = H * W          # 50176
NPP = NPIX // 128     # 392
FROWS = H - 1         # 223
SLOTS_PER_ROW = W // 2
NSLOT = FROWS * SLOTS_PER_ROW  # 24976

GCHUNKS = 28     # gather instructions per image (gen/drain granularity)
CGROUP = 2       # gather chunks per combine/store group
GTH_BUFS = 8     # combine-group tiles in flight


def set_knobs(gchunks=None, gth_bufs=None, cgroup=None):
    global GCHUNKS, GTH_BUFS, CGROUP
    if gchunks is not None:
        GCHUNKS = gchunks
    if gth_bufs is not None:
        GTH_BUFS = gth_bufs
    if cgroup is not None:
        CGROUP = cgroup


def build_module(n_imgs: int = NIMG) -> bass.Bass:
    idxc = NPIX // GCHUNKS
    cj = idxc // 128
    nc = bacc.Bacc(num_swdge_queues=4)

    x_t = nc.dram_tensor("x", [n_imgs, H, W, 5], F32, kind="ExternalInput")
    out_t = nc.dram_tensor("out", [n_imgs, H, W, 3], F32, kind="ExternalOutput")

    x_flat = x_t[:].rearrange("n h w c -> n (h w c)")
    out_flat = out_t[:].rearrange("n h w c -> n (h w c)").rearrange(
        "n (p f) -> n p f", p=128
    )

    with TileContext(nc) as tc:
        with (
            tc.tile_pool(name="raw", bufs=3) as raw_pool,
            tc.tile_pool(name="mid", bufs=1) as mid_pool,
            tc.tile_pool(name="slotp", bufs=2) as slot_pool,
            tc.tile_pool(name="tmp", bufs=1) as tmp_pool,
            tc.tile_pool(name="wv", bufs=2) as wv_pool,
            tc.tile_pool(name="k16p", bufs=2) as k16_pool,
            tc.tile_pool(name="k2p", bufs=2) as k2_pool,
            tc.tile_pool(name="widxp", bufs=2) as widx_pool,
            tc.tile_pool(name="wgt", bufs=2) as wgt_pool,
            tc.tile_pool(name="gth", bufs=GTH_BUFS) as gth_pool,
            tc.tile_pool(name="accp", bufs=3) as acc_pool,
            tc.tile_pool(name="dram", bufs=1, space="DRAM") as dram_pool,
        ):
            state = {}
            imf_all = dram_pool.tile([H * n_imgs, W * 3], F32, tag="imf")
            slotd_all = dram_pool.tile([NSLOT * n_imgs, 64], F32, tag="slotd")

            # Const tiles (built once; tensor_scalar and 2-port-capable DVE
            # ops are banned from the steady state — they can lock GPSIMD
            # out of the shared SBUF port and stall SWDGE descriptor gen).
            consts = tmp_pool.tile([128, 3 * NPP], F32, tag="consts")
            ones = consts[:, 0:NPP]
            half = consts[:, NPP:2 * NPP]
            big = consts[:, 2 * NPP:3 * NPP]   # 2^23: RNE rounding shifter
            nc.vector.memset(ones, 1.0)
            nc.vector.memset(half, 0.5)
            nc.vector.memset(big, 8388608.0)

            raws = {}

            def emit_raw(b):
                """Prefetch the raw image (sync ring carries only these)."""
                raw = raw_pool.tile([128, NPP * 5], F32)
                nc.sync.dma_start(
                    out=raw[:], in_=x_flat[b].rearrange("(p f) -> p f", p=128)
                )
                raws[b] = raw

            def emit_build(b):
                """Slot table + idxs + weights (scalar DMA + ACT + DVE)."""
                raw = raws.pop(b)
                rawv = raw[:].rearrange("p (q c) -> p q c", c=5)
                x_ap = rawv[:, :, 3:4].rearrange("p q c -> p (q c)")
                y_ap = rawv[:, :, 4:5].rearrange("p q c -> p (q c)")

                # --- floors, slot ids, weights.  Cast-free: DVE casts /
                # copies / tensor_scalar are 2-port ops that mutually block
                # GPSIMD's SWDGE descriptor generation, so floor is done
                # with the 2^23 RNE shifter trick (pure tensor_tensor) and
                # the i16 idx is the low half of (k + 2^23)'s bit pattern.
                def floor_tt(src_ap, nm):
                    r = tmp_pool.tile([128, NPP], F32, tag=f"r{nm}")
                    nc.vector.tensor_tensor(out=r[:], in0=src_ap, in1=big,
                                            op=OP.add)
                    nc.vector.tensor_tensor(out=r[:], in0=r[:], in1=big,
                                            op=OP.subtract)
                    gt = tmp_pool.tile([128, NPP], F32, tag=f"g{nm}")
                    nc.vector.tensor_tensor(out=gt[:], in0=r[:], in1=src_ap,
                                            op=OP.is_gt)
                    nc.vector.tensor_tensor(out=r[:], in0=r[:], in1=gt[:],
                                            op=OP.subtract)
                    return r

                fxf = floor_tt(x_ap, "x")
                fyf = floor_tt(y_ap, "y")

                wv = wv_pool.tile([128, 3 * NPP], F32, tag="wv")
                wx1 = wv[:, 0:NPP]
                wy1 = wv[:, NPP:2 * NPP]
                m1 = wv[:, 2 * NPP:3 * NPP]
                nc.vector.tensor_tensor(out=wx1, in0=x_ap, in1=fxf[:],
                                        op=OP.subtract)
                nc.vector.tensor_tensor(out=wy1, in0=y_ap, in1=fyf[:],
                                        op=OP.subtract)
                # fl2 = floor(fx/2);  sel = fx - 2*fl2;  k = fy*112 + fl2
                fh = tmp_pool.tile([128, NPP], F32, tag="fh")
                nc.vector.tensor_tensor(out=fh[:], in0=fxf[:], in1=half,
                                        op=OP.mult)
                fl2 = floor_tt(fh[:], "h")
                nc.vector.scalar_tensor_tensor(
                    out=m1, in0=fl2[:], scalar=-2.0, in1=fxf[:],
                    op0=OP.mult, op1=OP.add,
                )
                kf = tmp_pool.tile([128, NPP], F32, tag="kf")
                nc.vector.scalar_tensor_tensor(
                    out=kf[:], in0=fyf[:], scalar=float(W // 2), in1=fl2[:],
                    op0=OP.mult, op1=OP.add,
                )
                # kb = kf + 2^23: f32 bits 0x4B00_0000 | k, so the low i16
                # of each word IS k.  Compact on ACT (same-dtype copy).
                nc.vector.tensor_tensor(out=kf[:], in0=kf[:], in1=big,
                                        op=OP.add)
                kb16 = kf[:].bitcast(I16).rearrange("p (n two) -> p n two",
                                                    two=2)
                k16 = k16_pool.tile([128, NPP], I16, tag="k16")
                nc.scalar.copy(
                    out=k16[:].rearrange("p (n one) -> p n one", one=1),
                    in_=kb16[:, :, 0:1],
                )

                # --- idx wrap: widx[r, 8n+m] = k16[16m+r, n] ---
                # step 1: partition rebase, 8 contiguous SBUF->SBUF DMAs
                k2 = k2_pool.tile([128, NPP * 8], I16, tag="k2")
                for m in range(8):
                    nc.sync.dma_start(
                        out=k2[0:16, NPP * m:NPP * (m + 1)],
                        in_=k16[16 * m:16 * (m + 1), :],
                    )
                # step 2: free-dim interleave on DVE (lane-local)
                widxs = widx_pool.tile([128, NPP * 8], I16, tag="widxs")
                k2v = k2[0:16, :].rearrange("p (m n) -> p m n", m=8)
                wxv = widxs[0:16, :].rearrange("p (n m) -> p m n", m=8)
                nc.scalar.copy(out=wxv, in_=k2v)
                # step 3: replicate to all 8 core groups
                for g in (1, 2, 4):
                    nc.sync.dma_start(
                        out=widxs[16 * g:16 * min(2 * g, 8), :],
                        in_=widxs[0:16 * min(g, 8 - g), :],
                    )

                # --- image rows to DRAM; row r = imf[r], 672 f32 ---
                img_rows = mid_pool.tile([128, NPP * 3], F32, tag="deint")
                nc.scalar.copy(
                    out=img_rows[:].rearrange("p (q c) -> p q c", c=3),
                    in_=rawv[:, :, 0:3],
                )
                imf = imf_all[H * b:H * (b + 1), :]
                nc.scalar.dma_start(
                    out=imf.rearrange("h f -> (h f)").rearrange(
                        "(p f) -> p f", p=128
                    ),
                    in_=img_rows[:],
                )

                # --- 256B-slot table (2-parity, as baseline) ---
                slotd = slotd_all[NSLOT * b:NSLOT * (b + 1), :]
                slotd_rows = slotd.rearrange(
                    "(r s) w -> r (s w)", s=SLOTS_PER_ROW
                )
                for h in range(2):
                    r0 = 128 * h
                    nrow = 128 if h == 0 else FROWS - 128
                    pp = mid_pool.tile([128, 1344], F32, tag="pp")
                    nc.scalar.dma_start(out=pp[0:nrow, 0:672],
                                        in_=imf[r0:r0 + nrow])
                    nc.scalar.dma_start(
                        out=pp[0:nrow, 672:1344], in_=imf[r0 + 1:r0 + nrow + 1]
                    )
                    frow = mid_pool.tile([128, 1350], F32, tag="frow")
                    nc.scalar.copy(
                        out=frow[0:nrow, 0:1344].rearrange(
                            "p (xx k c) -> p xx k c", k=2, c=3
                        ),
                        in_=pp[0:nrow].rearrange(
                            "p (k xx c) -> p xx k c", k=2, c=3
                        ),
                    )
                    nc.scalar.memzero(frow[0:nrow, 1344:1350])
                    half_s = SLOTS_PER_ROW // 2
                    slotd_cols = slotd_rows[r0:r0 + nrow].rearrange(
                        "r (s w) -> r s w", w=64
                    )
                    for sc in range(2):
                        slotbuf = slot_pool.tile([128, half_s * 64], F32,
                                                 tag="slotbuf")
                        fr_ap = frow[0:nrow]
                        slot_src = bass.AP(
                            fr_ap.tensor,
                            fr_ap.offset + sc * half_s * 12,
                            [list(fr_ap.ap[0]), [12, half_s], [1, 18]],
                        )
                        nc.scalar.copy(
                            out=slotbuf[0:nrow].rearrange(
                                "p (s w) -> p s w", w=64
                            )[:, :, 0:18],
                            in_=slot_src,
                        )
                        nc.scalar.dma_start(
                            out=slotd_cols[:, sc * half_s:(sc + 1) * half_s, :],
                            in_=slotbuf[0:nrow],
                        )

                # --- weight expansion (DVE, tensor_tensor only) ---
                wx0 = tmp_pool.tile([128, NPP], F32, tag="wx0")
                nc.vector.tensor_tensor(out=wx0[:], in0=ones, in1=wx1,
                                        op=OP.subtract)
                wy0 = tmp_pool.tile([128, NPP], F32, tag="wy0")
                nc.vector.tensor_tensor(out=wy0[:], in0=ones, in1=wy1,
                                        op=OP.subtract)
                m0 = tmp_pool.tile([128, NPP], F32, tag="m0")
                nc.vector.tensor_tensor(out=m0[:], in0=ones, in1=m1,
                                        op=OP.subtract)
                u0 = tmp_pool.tile([128, NPP], F32, tag="u0")
                nc.vector.tensor_tensor(out=u0[:], in0=m0[:], in1=wx0[:],
                                        op=OP.mult)
                u2 = tmp_pool.tile([128, NPP], F32, tag="u2")
                nc.vector.tensor_tensor(out=u2[:], in0=m1, in1=wx1,
                                        op=OP.mult)
                u1a = tmp_pool.tile([128, NPP], F32, tag="rh")
                nc.vector.tensor_tensor(out=u1a[:], in0=m0[:], in1=wx1,
                                        op=OP.mult)
                u1b = tmp_pool.tile([128, NPP], F32, tag="gh")
                nc.vector.tensor_tensor(out=u1b[:], in0=m1, in1=wx0[:],
                                        op=OP.mult)
                u1 = tmp_pool.tile([128, NPP], F32, tag="u1")
                nc.vector.tensor_tensor(out=u1[:], in0=u1a[:], in1=u1b[:],
                                        op=OP.add)
                wmat = []
                for v, uv in enumerate((u0, u1, u2)):
                    row = []
                    for ky, wyk in enumerate((wy0, wy1)):
                        wt = wgt_pool.tile([128, NPP], F32, tag=f"w{v}{ky}")
                        nc.vector.tensor_tensor(out=wt[:], in0=uv[:],
                                                in1=wyk[:], op=OP.mult)
                        row.append(wt)
                    wmat.append(row)
                state[b] = {"slotd": slotd, "widxs": widxs, "wmat": wmat}

            def emit_gather(b):
                st = state[b]
                slotd, widxs, wmat = st["slotd"], st["widxs"], st["wmat"]
                ngrp = GCHUNKS // CGROUP
                gj = cj * CGROUP  # slots per combine group
                for grp in range(ngrp):
                    gth = gth_pool.tile([128, gj * 64], F32, tag="gth")
                    for sub in range(CGROUP):
                        ck = grp * CGROUP + sub
                        nc.gpsimd.dma_gather(
                            out_ap=gth[:, cj * 64 * sub:cj * 64 * (sub + 1)]
                            .rearrange("p (n w) -> p n w", w=64),
                            in_ap=slotd,
                            idxs_ap=widxs[:, (idxc // 16) * ck:
                                          (idxc // 16) * (ck + 1)],
                            num_idxs=idxc,
                            num_idxs_reg=idxc,
                            elem_size=64,
                            single_packet=False,
                            queue_num=ck % 4,
                        )
                    gv = gth[:].rearrange("p (n w) -> p n w", w=64)
                    acc = acc_pool.tile([128, gj * 3], F32, tag="acc")
                    accv = acc[:].rearrange("p (q c) -> p q c", c=3)
                    tmp = acc_pool.tile([128, gj * 3], F32, tag="tmp")
                    tmpv = tmp[:].rearrange("p (q c) -> p q c", c=3)
                    first = True
                    for v in range(3):
                        for ky in range(2):
                            wap = (
                                wmat[v][ky][:, gj * grp: gj * (grp + 1)]
                                .unsqueeze(2)
                                .broadcast_to([128, gj, 3])
                            )
                            dst = accv if first else tmpv
                            off = 6 * v + 3 * ky
                            nc.vector.tensor_tensor(
                                out=dst, in0=gv[:, :, off:off + 3], in1=wap,
                                op=OP.mult,
                            )
                            if not first:
                                nc.vector.tensor_tensor(
                                    out=accv, in0=accv, in1=tmpv, op=OP.add
                                )
                            first = False
                    nc.scalar.dma_start(
                        out=out_flat[b][:, gj * 3 * grp: gj * 3 * (grp + 1)],
                        in_=acc[:],
                    )
                del state[b]

            emit_raw(0)
            if n_imgs > 1:
                emit_raw(1)
            emit_build(0)
            for b in range(n_imgs):
                if b + 2 < n_imgs:
                    emit_raw(b + 2)
                if b + 1 < n_imgs:
                    emit_build(b + 1)
                emit_gather(b)

    nc.compile()
    return nc


def kernel(x: np.ndarray) -> np.ndarray:
    """Full-input entry point: shards the batch over 8 NeuronCores."""
    from concourse import bass_utils

    B = x.shape[0]
    assert x.shape == (B, H, W, 5) and B % NCORES == 0
    per = B // NCORES
    nc = build_module(per)
    in_maps = [
        {"x": np.ascontiguousarray(x[c * per:(c + 1) * per])}
        for c in range(NCORES)
    ]
    res = bass_utils.run_bass_kernel_spmd(nc, in_maps, core_ids=list(range(NCORES)))
    out = np.concatenate([res.results[c]["out"] for c in range(NCORES)], axis=0)
    return out
